# revision 3
# baseline (speedup 1.0000x reference)
"""Trainium2 Bass kernel for hierarchical softmax tree posterior (HNet.predict).

Math: per internal node i (level-order, children 2i+1/2i+2), softmax over 2
children of Linear(x). Path probabilities multiply down a depth-12 complete
binary tree; output p [B, 4096] leaf posteriors.

Key identities used:
  softmax([l0, l1])[0] = sigmoid(l0 - l1), [1] = 1 - sigmoid(l0 - l1)
  => only the logit DIFFERENCE matters: d_j = x . (W_j0 - W_j1) + (b_j0 - b_j1)
  => one [B,64] @ [64,4095] matmul (with bias folded in as a 65th contraction
     row), sigmoid on ScalarE, then the multiply-down-the-tree on VectorE:
     child0 = p * s, child1 = p - child0.

Sharding: batch B=8192 split across 8 cores (1024 rows each); the small
tree parameters are replicated. Output [B, 4096] f32 = 128MB dominates
traffic (memory-bound): each core writes 16.8MB.
"""

import numpy as np

import concourse.bacc as bacc
import concourse.mybir as mybir
import concourse.tile as tile
from concourse.bass_utils import run_bass_kernel_spmd

B, D = 8192, 64
NODES = 4095          # internal nodes, level-order
LEAVES = 4096
NCORES = 8
BLOC = B // NCORES    # 1024 rows per core
KA = D + 1            # contraction dim incl. bias row
NBT = BLOC // 128     # 8 batch tiles of 128 rows
GROUPS = 2            # phase-A batch-tile groups (fused small-level tree ops)
GBT = NBT // GROUPS   # 4 batch tiles per group

F32 = mybir.dt.float32
# float32r runs the PE at 1 cyc/row (vs 4 for exact fp32); measured matmul
# error 1.7e-4 rel-to-scale which stays ~1e-3 in the final posteriors.
MM_DT = mybir.dt.float32r

# Last-level odd children (p - p*s) computed on GPSIMD to unload the DVE,
# which is otherwise the bottleneck engine (fp32 tensor_tensor is 1x rate).
SUB11_ON_GPSIMD = True


def _build(trace_label=""):
    nc = bacc.Bacc("TRN2", target_bir_lowering=False, debug=False, num_devices=NCORES)
    wdt = nc.dram_tensor("wdt", [KA, LEAVES], F32, kind="ExternalInput")
    xt = nc.dram_tensor("xt", [KA, BLOC], F32, kind="ExternalInput")
    out = nc.dram_tensor("out", [BLOC, LEAVES], F32, kind="ExternalOutput")

    SIG = mybir.ActivationFunctionType.Sigmoid
    IDN = mybir.ActivationFunctionType.Identity

    with tile.TileContext(nc) as tc:
        with (
            tc.tile_pool(name="const", bufs=1) as const,
            tc.tile_pool(name="pa", bufs=1) as pa,
            tc.tile_pool(name="p10p", bufs=2) as p10p,
            tc.tile_pool(name="pb", bufs=2) as pb,
            tc.tile_pool(name="ps", bufs=2, space="PSUM") as psp,
        ):
            wdt_sb = const.tile([KA, LEAVES], F32)
            xt_sb = const.tile([KA, BLOC], F32)
            nc.sync.dma_start(out=wdt_sb[:], in_=wdt[:])
            nc.sync.dma_start(out=xt_sb[:], in_=xt[:])
            if MM_DT != F32:
                wdt_r = const.tile([KA, LEAVES], MM_DT)
                xt_r = const.tile([KA, BLOC], MM_DT)
                nc.vector.tensor_copy(wdt_r[:], wdt_sb[:])
                nc.vector.tensor_copy(xt_r[:], xt_sb[:])
            else:
                wdt_r, xt_r = wdt_sb, xt_sb

            for g in range(GROUPS):
                bts = range(g * GBT, (g + 1) * GBT)
                # ---- phase A: nodes 0..1022 (levels 0..9) for GBT batch tiles
                s_small = pa.tile([128, GBT, 1024], F32, tag="s_small")
                for j, bt in enumerate(bts):
                    ps = psp.tile([128, 1024], F32, tag="ps")
                    for c in range(2):
                        nc.tensor.matmul(
                            ps[:, c * 512:(c + 1) * 512],
                            xt_r[:, bt * 128:(bt + 1) * 128],
                            wdt_r[:, c * 512:(c + 1) * 512],
                            start=True, stop=True,
                        )
                    nc.scalar.activation(out=s_small[:, j, :], in_=ps[:], func=SIG)

                # fused small-level tree over the group: p tiles [128, GBT, n]
                pA = pa.tile([128, GBT, 512], F32, tag="pA")
                pB = pa.tile([128, GBT, 512], F32, tag="pB")
                p10 = p10p.tile([128, GBT, 1024], F32, tag="p10")
                # level 0: p1 = [s0, 1-s0]
                nc.vector.tensor_copy(pA[:, :, 0:1], s_small[:, :, 0:1])
                nc.scalar.activation(out=pA[:, :, 1:2], in_=s_small[:, :, 0:1],
                                     func=IDN, bias=1.0, scale=-1.0)
                cur = pA
                other = pB
                for lvl in range(1, 10):
                    n = 1 << lvl
                    off = n - 1
                    nxt = p10 if lvl == 9 else other
                    nxt4 = nxt[:, :, 0:2 * n].rearrange("p g (n two) -> p g n two", two=2)
                    nc.vector.tensor_mul(nxt4[:, :, :, 0], cur[:, :, 0:n],
                                         s_small[:, :, off:off + n])
                    nc.vector.tensor_sub(nxt4[:, :, :, 1], cur[:, :, 0:n],
                                         nxt4[:, :, :, 0])
                    other = cur
                    cur = nxt

                # ---- phase B: nodes 1023..4094 (levels 10..11), per batch tile
                for j, bt in enumerate(bts):
                    ps1 = psp.tile([128, 2048], F32, tag="ps")   # nodes 1023..3070
                    for c in range(4):
                        nc.tensor.matmul(
                            ps1[:, c * 512:(c + 1) * 512],
                            xt_r[:, bt * 128:(bt + 1) * 128],
                            wdt_r[:, 1023 + c * 512:1023 + (c + 1) * 512],
                            start=True, stop=True,
                        )
                    ps2 = psp.tile([128, 1024], F32, tag="ps")   # nodes 3071..4094
                    for c in range(2):
                        nc.tensor.matmul(
                            ps2[:, c * 512:(c + 1) * 512],
                            xt_r[:, bt * 128:(bt + 1) * 128],
                            wdt_r[:, 3071 + c * 512:3071 + (c + 1) * 512],
                            start=True, stop=True,
                        )
                    sb = pb.tile([128, 3072], F32, tag="sbig")
                    nc.scalar.activation(out=sb[:, 0:2048], in_=ps1[:], func=SIG)
                    nc.scalar.activation(out=sb[:, 2048:3072], in_=ps2[:], func=SIG)

                    # level 10: p10 [*,1024] -> p11 [*,2048]; s nodes 1023..2046
                    p11 = pb.tile([128, 2048], F32, tag="p11")
                    p11v = p11.rearrange("p (n two) -> p n two", two=2)
                    nc.vector.tensor_mul(p11v[:, :, 0], p10[:, j, :], sb[:, 0:1024])
                    nc.vector.tensor_sub(p11v[:, :, 1], p10[:, j, :], p11v[:, :, 0])

                    # level 11: p11 -> out tile [*,4096]; s nodes 2047..4094
                    ot = pb.tile([128, 4096], F32, tag="out")
                    otv = ot.rearrange("p (n two) -> p n two", two=2)
                    nc.vector.tensor_mul(otv[:, :, 0], p11[:], sb[:, 1024:3072])
                    if SUB11_ON_GPSIMD:
                        nc.gpsimd.tensor_sub(otv[:, :, 1], p11[:], otv[:, :, 0])
                    else:
                        nc.vector.tensor_sub(otv[:, :, 1], p11[:], otv[:, :, 0])

                    nc.sync.dma_start(out=out[bt * 128:(bt + 1) * 128, :], in_=ot[:])

    nc.compile()
    return nc


_NC_CACHE = {}


def _get_nc():
    if "nc" not in _NC_CACHE:
        _NC_CACHE["nc"] = _build()
    return _NC_CACHE["nc"]


def _prep_inputs(x, W, b):
    x = np.asarray(x, dtype=np.float32)
    W = np.asarray(W, dtype=np.float32)
    b = np.asarray(b, dtype=np.float32)
    Wd = W[:, 0, :] - W[:, 1, :]          # [4095, 64]
    bd = b[:, 0] - b[:, 1]                # [4095]
    wdt = np.zeros((KA, LEAVES), dtype=np.float32)
    wdt[:D, :NODES] = Wd.T
    wdt[D, :NODES] = bd
    xt = np.empty((KA, B), dtype=np.float32)
    xt[:D] = x.T
    xt[D] = 1.0
    in_maps = [
        {"wdt": wdt, "xt": np.ascontiguousarray(xt[:, c * BLOC:(c + 1) * BLOC])}
        for c in range(NCORES)
    ]
    return in_maps


def kernel(x, W, b, _trace=False, _trace_kwargs=None):
    in_maps = _prep_inputs(x, W, b)
    nc = _get_nc()
    res = run_bass_kernel_spmd(
        nc, in_maps, core_ids=list(range(NCORES)),
        trace=_trace, **(_trace_kwargs or {}),
    )
    full = np.concatenate([res.results[c]["out"] for c in range(NCORES)], axis=0)
    if _trace:
        return full, res
    return full


if __name__ == "__main__":
    rng = np.random.default_rng(0)
    x = rng.standard_normal((B, D)).astype(np.float32)
    W = (rng.standard_normal((NODES, 2, D)) * 0.1).astype(np.float32)
    b = (rng.standard_normal((NODES, 2)) * 0.1).astype(np.float32)
    p = kernel(x, W, b)
    print("out", p.shape, p.dtype, "rowsum", p.sum(axis=1)[:4])


# revision 4
# speedup vs baseline: 1.1472x; 1.1472x over previous
"""Trainium2 Bass kernel for hierarchical softmax tree posterior (HNet.predict).

Math: per internal node i (level-order, children 2i+1/2i+2), softmax over 2
children of Linear(x). Path probabilities multiply down a depth-12 complete
binary tree; output p [B, 4096] leaf posteriors.

Key identities used:
  softmax([l0, l1])[0] = sigmoid(l0 - l1), [1] = 1 - sigmoid(l0 - l1)
  => only the logit DIFFERENCE matters: d_j = x . (W_j0 - W_j1) + (b_j0 - b_j1)
  => one [B,64] @ [64,4095] matmul (with bias folded in as a 65th contraction
     row), sigmoid on ScalarE, then the multiply-down-the-tree on VectorE:
     child0 = p * s, child1 = p - child0.

Sharding: batch B=8192 split across 8 cores (1024 rows each); the small
tree parameters are replicated. Output [B, 4096] f32 = 128MB dominates
traffic (memory-bound): each core writes 16.8MB.
"""

import numpy as np

import concourse.bacc as bacc
import concourse.mybir as mybir
import concourse.tile as tile
from concourse.bass_utils import run_bass_kernel_spmd

B, D = 8192, 64
NODES = 4095          # internal nodes, level-order
LEAVES = 4096
NCORES = 8
BLOC = B // NCORES    # 1024 rows per core
KA = D + 1            # contraction dim incl. bias row
NBT = BLOC // 128     # 8 batch tiles of 128 rows
GROUPS = 2            # phase-A batch-tile groups (fused small-level tree ops)
GBT = NBT // GROUPS   # 4 batch tiles per group

F32 = mybir.dt.float32
# float32r runs the PE at 1 cyc/row (vs 4 for exact fp32); measured matmul
# error 1.7e-4 rel-to-scale which stays ~1e-3 in the final posteriors.
MM_DT = mybir.dt.float32r

# Last-level odd children (p - p*s) computed on GPSIMD to unload the DVE,
# which is otherwise the bottleneck engine (fp32 tensor_tensor is 1x rate).
SUB11_ON_GPSIMD = True


def _build(reps=1):
    nc = bacc.Bacc("TRN2", target_bir_lowering=False, debug=False, num_devices=NCORES)
    wdt = nc.dram_tensor("wdt", [KA, LEAVES], F32, kind="ExternalInput")
    xt = nc.dram_tensor("xt", [KA, BLOC], F32, kind="ExternalInput")
    out = nc.dram_tensor("out", [BLOC, LEAVES], F32, kind="ExternalOutput")

    SIG = mybir.ActivationFunctionType.Sigmoid
    IDN = mybir.ActivationFunctionType.Identity

    with tile.TileContext(nc) as tc:
        with (
            tc.tile_pool(name="const", bufs=1) as const,
            tc.tile_pool(name="pa", bufs=1) as pa,
            tc.tile_pool(name="p10p", bufs=2) as p10p,
            tc.tile_pool(name="pb", bufs=2) as pb,
            tc.tile_pool(name="ps", bufs=2, space="PSUM") as psp,
        ):
            wdt_sb = const.tile([KA, LEAVES], F32)
            xt_sb = const.tile([KA, BLOC], F32)
            nc.sync.dma_start(out=wdt_sb[:], in_=wdt[:])
            nc.sync.dma_start(out=xt_sb[:], in_=xt[:])
            if MM_DT != F32:
                wdt_r = const.tile([KA, LEAVES], MM_DT)
                xt_r = const.tile([KA, BLOC], MM_DT)
                nc.vector.tensor_copy(wdt_r[:], wdt_sb[:])
                nc.vector.tensor_copy(xt_r[:], xt_sb[:])
            else:
                wdt_r, xt_r = wdt_sb, xt_sb

            import contextlib
            loop_ctx = tc.For_i(0, reps, 1) if reps > 1 else contextlib.nullcontext()
            with loop_ctx:
                _emit_body(nc, tc, pa, p10p, pb, psp, wdt_r, xt_r, out, SIG, IDN)

    nc.compile()
    return nc


def _emit_body(nc, tc, pa, p10p, pb, psp, wdt_r, xt_r, out, SIG, IDN):
    F32 = mybir.dt.float32
    if True:
            for g in range(GROUPS):
                bts = range(g * GBT, (g + 1) * GBT)
                # ---- phase A: nodes 0..1022 (levels 0..9) for GBT batch tiles
                s_small = pa.tile([128, GBT, 1024], F32, tag="s_small")
                for j, bt in enumerate(bts):
                    ps = psp.tile([128, 1024], F32, tag="ps")
                    for c in range(2):
                        nc.tensor.matmul(
                            ps[:, c * 512:(c + 1) * 512],
                            xt_r[:, bt * 128:(bt + 1) * 128],
                            wdt_r[:, c * 512:(c + 1) * 512],
                            start=True, stop=True,
                        )
                    nc.scalar.activation(out=s_small[:, j, :], in_=ps[:], func=SIG)

                # fused small-level tree over the group: p tiles [128, GBT, n]
                pA = pa.tile([128, GBT, 512], F32, tag="pA")
                pB = pa.tile([128, GBT, 512], F32, tag="pB")
                p10 = p10p.tile([128, GBT, 1024], F32, tag="p10")
                # level 0: p1 = [s0, 1-s0]
                nc.vector.tensor_copy(pA[:, :, 0:1], s_small[:, :, 0:1])
                nc.scalar.activation(out=pA[:, :, 1:2], in_=s_small[:, :, 0:1],
                                     func=IDN, bias=1.0, scale=-1.0)
                cur = pA
                other = pB
                for lvl in range(1, 10):
                    n = 1 << lvl
                    off = n - 1
                    nxt = p10 if lvl == 9 else other
                    nxt4 = nxt[:, :, 0:2 * n].rearrange("p g (n two) -> p g n two", two=2)
                    nc.vector.tensor_mul(nxt4[:, :, :, 0], cur[:, :, 0:n],
                                         s_small[:, :, off:off + n])
                    nc.vector.tensor_sub(nxt4[:, :, :, 1], cur[:, :, 0:n],
                                         nxt4[:, :, :, 0])
                    other = cur
                    cur = nxt

                # ---- phase B: nodes 1023..4094 (levels 10..11), per batch tile
                for j, bt in enumerate(bts):
                    ps1 = psp.tile([128, 2048], F32, tag="ps")   # nodes 1023..3070
                    for c in range(4):
                        nc.tensor.matmul(
                            ps1[:, c * 512:(c + 1) * 512],
                            xt_r[:, bt * 128:(bt + 1) * 128],
                            wdt_r[:, 1023 + c * 512:1023 + (c + 1) * 512],
                            start=True, stop=True,
                        )
                    ps2 = psp.tile([128, 1024], F32, tag="ps")   # nodes 3071..4094
                    for c in range(2):
                        nc.tensor.matmul(
                            ps2[:, c * 512:(c + 1) * 512],
                            xt_r[:, bt * 128:(bt + 1) * 128],
                            wdt_r[:, 3071 + c * 512:3071 + (c + 1) * 512],
                            start=True, stop=True,
                        )
                    sb = pb.tile([128, 3072], F32, tag="sbig")
                    nc.scalar.activation(out=sb[:, 0:2048], in_=ps1[:], func=SIG)
                    nc.scalar.activation(out=sb[:, 2048:3072], in_=ps2[:], func=SIG)

                    # level 10: p10 [*,1024] -> p11 [*,2048]; s nodes 1023..2046
                    p11 = pb.tile([128, 2048], F32, tag="p11")
                    p11v = p11.rearrange("p (n two) -> p n two", two=2)
                    nc.vector.tensor_mul(p11v[:, :, 0], p10[:, j, :], sb[:, 0:1024])
                    nc.vector.tensor_sub(p11v[:, :, 1], p10[:, j, :], p11v[:, :, 0])

                    # level 11: p11 -> out tile [*,4096]; s nodes 2047..4094
                    ot = pb.tile([128, 4096], F32, tag="out")
                    otv = ot.rearrange("p (n two) -> p n two", two=2)
                    nc.vector.tensor_mul(otv[:, :, 0], p11[:], sb[:, 1024:3072])
                    if SUB11_ON_GPSIMD:
                        nc.gpsimd.tensor_sub(otv[:, :, 1], p11[:], otv[:, :, 0])
                    else:
                        nc.vector.tensor_sub(otv[:, :, 1], p11[:], otv[:, :, 0])

                    nc.sync.dma_start(out=out[bt * 128:(bt + 1) * 128, :], in_=ot[:])


_NC_CACHE = {}


def _get_nc(reps=1):
    if reps not in _NC_CACHE:
        _NC_CACHE[reps] = _build(reps)
    return _NC_CACHE[reps]


def _prep_inputs(x, W, b):
    x = np.asarray(x, dtype=np.float32)
    W = np.asarray(W, dtype=np.float32)
    b = np.asarray(b, dtype=np.float32)
    Wd = W[:, 0, :] - W[:, 1, :]          # [4095, 64]
    bd = b[:, 0] - b[:, 1]                # [4095]
    wdt = np.zeros((KA, LEAVES), dtype=np.float32)
    wdt[:D, :NODES] = Wd.T
    wdt[D, :NODES] = bd
    xt = np.empty((KA, B), dtype=np.float32)
    xt[:D] = x.T
    xt[D] = 1.0
    in_maps = [
        {"wdt": wdt, "xt": np.ascontiguousarray(xt[:, c * BLOC:(c + 1) * BLOC])}
        for c in range(NCORES)
    ]
    return in_maps


def kernel(x, W, b, _trace=False, _trace_kwargs=None):
    in_maps = _prep_inputs(x, W, b)
    nc = _get_nc()
    res = run_bass_kernel_spmd(
        nc, in_maps, core_ids=list(range(NCORES)),
        trace=_trace, **(_trace_kwargs or {}),
    )
    full = np.concatenate([res.results[c]["out"] for c in range(NCORES)], axis=0)
    if _trace:
        return full, res
    return full


if __name__ == "__main__":
    rng = np.random.default_rng(0)
    x = rng.standard_normal((B, D)).astype(np.float32)
    W = (rng.standard_normal((NODES, 2, D)) * 0.1).astype(np.float32)
    b = (rng.standard_normal((NODES, 2)) * 0.1).astype(np.float32)
    p = kernel(x, W, b)
    print("out", p.shape, p.dtype, "rowsum", p.sum(axis=1)[:4])


# revision 9
# speedup vs baseline: 1.9242x; 1.6773x over previous
"""Trainium2 Bass kernel for hierarchical softmax tree posterior (HNet.predict).

Math: per internal node i (level-order, children 2i+1/2i+2), softmax over 2
children of Linear(x). Path probabilities multiply down a depth-12 complete
binary tree; output p [B, 4096] leaf posteriors.

Key identities used:
  softmax([l0, l1])[0] = sigmoid(l0 - l1), [1] = 1 - sigmoid(l0 - l1)
  => only the logit DIFFERENCE matters: d_j = x . (W_j0 - W_j1) + (b_j0 - b_j1)
  => one [B,64] @ [64,4095] matmul (bias folded in as a 65th contraction row),
     sigmoid on ScalarE, then multiply-down-the-tree on VectorE:
     child0 = p * s, child1 = p - child0.
     (GPSIMD offload of subtractions was measured NET-NEGATIVE: it shares an
     SBUF port with VectorE and serializes; TensorTensor cannot run on
     ScalarE on TRN2 — so the whole tree stays on the DVE.)

Sharding: batch B=8192 split across 8 cores (1024 rows each); tree params
replicated. Output [B, 4096] f32 = 128MB dominates traffic (memory-bound).
"""

import contextlib

import numpy as np

import concourse.bacc as bacc
import concourse.mybir as mybir
import concourse.tile as tile
from concourse.bass_utils import run_bass_kernel_spmd

B, D = 8192, 64
NODES = 4095          # internal nodes, level-order
LEAVES = 4096
NCORES = 8
BLOC = B // NCORES    # 1024 rows per core
KA = D + 1            # contraction dim incl. bias row
NBT = BLOC // 128     # 8 batch tiles of 128 rows

F32 = mybir.dt.float32
# float32r runs the PE at 1 cyc/row (vs 4 for exact fp32); measured end-to-end
# output error 2.4e-4 rel-to-scale. DRAM inputs are declared float32r directly
# (same bytes as f32) so no on-device cast is needed.
MM_DT = mybir.dt.float32r

# Pair-columns of the level-10/11 odd-child subtractions on GPSIMD instead of
# VectorE. Measured on HW: any GPSIMD share is slower (shared SBUF port with
# DVE serializes the engines), so these stay 0.
GP_SUB10 = 0      # of 1024
GP_SUB11 = 0      # of 2048


def _build(reps=1):
    nc = bacc.Bacc("TRN2", target_bir_lowering=False, debug=False, num_devices=NCORES)
    wdt = nc.dram_tensor("wdt", [KA, LEAVES], MM_DT, kind="ExternalInput")
    xt = nc.dram_tensor("xt", [KA, BLOC], MM_DT, kind="ExternalInput")
    out = nc.dram_tensor("out", [BLOC, LEAVES], F32, kind="ExternalOutput")

    SIG = mybir.ActivationFunctionType.Sigmoid
    IDN = mybir.ActivationFunctionType.Identity

    with tile.TileContext(nc) as tc:
        with (
            tc.tile_pool(name="const", bufs=1) as const,
            tc.tile_pool(name="pa", bufs=1) as pa,
            tc.tile_pool(name="pb", bufs=2) as pb,
            tc.tile_pool(name="ps", bufs=2, space="PSUM") as psp,
        ):
            wdt_r = const.tile([KA, LEAVES], MM_DT)
            xt_r = const.tile([KA, BLOC], MM_DT)
            nc.sync.dma_start(out=wdt_r[:], in_=wdt[:])
            nc.sync.dma_start(out=xt_r[:], in_=xt[:])

            loop = tc.For_i(0, reps, 1) if reps > 1 else contextlib.nullcontext()
            with loop:
                _emit_body(nc, tc, pa, pb, psp, wdt_r, xt_r, out, SIG, IDN)

    nc.compile()
    return nc


def _emit_body(nc, tc, pa, pb, psp, wdt_r, xt_r, out, SIG, IDN):
    # ---- phase A: nodes 0..1022 (levels 0..9) fused across all 8 batch tiles
    s_small = pa.tile([128, NBT, 1024], F32, tag="s_small")
    for bt in range(NBT):
        ps = psp.tile([128, 1024], F32, tag="ps")
        for c in range(2):
            nc.tensor.matmul(
                ps[:, c * 512:(c + 1) * 512],
                xt_r[:, bt * 128:(bt + 1) * 128],
                wdt_r[:, c * 512:(c + 1) * 512],
                start=True, stop=True,
            )
        nc.scalar.activation(out=s_small[:, bt, :], in_=ps[:], func=SIG)

    pA = pa.tile([128, NBT, 512], F32, tag="pA")
    pB = pa.tile([128, NBT, 512], F32, tag="pB")
    p10 = pa.tile([128, NBT, 1024], F32, tag="p10")
    # level 0: p1 = [s0, 1-s0]
    nc.vector.tensor_copy(pA[:, :, 0:1], s_small[:, :, 0:1])
    nc.scalar.activation(out=pA[:, :, 1:2], in_=s_small[:, :, 0:1],
                         func=IDN, bias=1.0, scale=-1.0)
    cur, other = pA, pB
    for lvl in range(1, 10):
        n = 1 << lvl
        off = n - 1
        nxt = p10 if lvl == 9 else other
        nxt4 = nxt[:, :, 0:2 * n].rearrange("p g (n two) -> p g n two", two=2)
        nc.vector.tensor_mul(nxt4[:, :, :, 0], cur[:, :, 0:n],
                             s_small[:, :, off:off + n])
        nc.vector.tensor_sub(nxt4[:, :, :, 1], cur[:, :, 0:n],
                             nxt4[:, :, :, 0])
        other, cur = cur, nxt

    # ---- phase B: nodes 1023..4094 (levels 10..11), per batch tile
    for bt in range(NBT):
        ps1 = psp.tile([128, 2048], F32, tag="ps")   # nodes 1023..3070
        for c in range(4):
            nc.tensor.matmul(
                ps1[:, c * 512:(c + 1) * 512],
                xt_r[:, bt * 128:(bt + 1) * 128],
                wdt_r[:, 1023 + c * 512:1023 + (c + 1) * 512],
                start=True, stop=True,
            )
        ps2 = psp.tile([128, 1024], F32, tag="ps")   # nodes 3071..4094
        for c in range(2):
            nc.tensor.matmul(
                ps2[:, c * 512:(c + 1) * 512],
                xt_r[:, bt * 128:(bt + 1) * 128],
                wdt_r[:, 3071 + c * 512:3071 + (c + 1) * 512],
                start=True, stop=True,
            )
        sb = pb.tile([128, 3072], F32, tag="sbig")
        nc.scalar.activation(out=sb[:, 0:2048], in_=ps1[:], func=SIG)
        nc.scalar.activation(out=sb[:, 2048:3072], in_=ps2[:], func=SIG)

        # level 10: p10 [*,1024] -> p11 [*,2048]; s nodes 1023..2046
        p11 = pb.tile([128, 2048], F32, tag="p11")
        p11v = p11.rearrange("p (n two) -> p n two", two=2)
        nc.vector.tensor_mul(p11v[:, :, 0], p10[:, bt, :], sb[:, 0:1024])
        c10 = 1024 - GP_SUB10
        if c10 > 0:
            nc.vector.tensor_sub(p11v[:, 0:c10, 1], p10[:, bt, 0:c10],
                                 p11v[:, 0:c10, 0])
        if GP_SUB10 > 0:
            nc.gpsimd.tensor_sub(p11v[:, c10:1024, 1], p10[:, bt, c10:1024],
                                 p11v[:, c10:1024, 0])

        # level 11: p11 -> out tile [*,4096]; s nodes 2047..4094
        ot = pb.tile([128, 4096], F32, tag="out")
        otv = ot.rearrange("p (n two) -> p n two", two=2)
        nc.vector.tensor_mul(otv[:, :, 0], p11[:], sb[:, 1024:3072])
        c11 = 2048 - GP_SUB11
        if c11 > 0:
            nc.vector.tensor_sub(otv[:, 0:c11, 1], p11[:, 0:c11],
                                 otv[:, 0:c11, 0])
        if GP_SUB11 > 0:
            nc.gpsimd.tensor_sub(otv[:, c11:2048, 1], p11[:, c11:2048],
                                 otv[:, c11:2048, 0])

        nc.sync.dma_start(out=out[bt * 128:(bt + 1) * 128, :], in_=ot[:])


_NC_CACHE = {}


def _get_nc(reps=1):
    if reps not in _NC_CACHE:
        _NC_CACHE[reps] = _build(reps)
    return _NC_CACHE[reps]


def _prep_inputs(x, W, b):
    x = np.asarray(x, dtype=np.float32)
    W = np.asarray(W, dtype=np.float32)
    b = np.asarray(b, dtype=np.float32)
    Wd = W[:, 0, :] - W[:, 1, :]          # [4095, 64]
    bd = b[:, 0] - b[:, 1]                # [4095]
    wdt = np.zeros((KA, LEAVES), dtype=np.float32)
    wdt[:D, :NODES] = Wd.T
    wdt[D, :NODES] = bd
    xt = np.empty((KA, B), dtype=np.float32)
    xt[:D] = x.T
    xt[D] = 1.0
    in_maps = [
        {"wdt": wdt, "xt": np.ascontiguousarray(xt[:, c * BLOC:(c + 1) * BLOC])}
        for c in range(NCORES)
    ]
    return in_maps


def kernel(x, W, b):
    in_maps = _prep_inputs(x, W, b)
    nc = _get_nc()
    res = run_bass_kernel_spmd(nc, in_maps, core_ids=list(range(NCORES)))
    return np.concatenate([res.results[c]["out"] for c in range(NCORES)], axis=0)


if __name__ == "__main__":
    rng = np.random.default_rng(0)
    x = rng.standard_normal((B, D)).astype(np.float32)
    W = (rng.standard_normal((NODES, 2, D)) * 0.1).astype(np.float32)
    b = (rng.standard_normal((NODES, 2)) * 0.1).astype(np.float32)
    p = kernel(x, W, b)
    print("out", p.shape, p.dtype, "rowsum", p.sum(axis=1)[:4])
